# revision 33
# baseline (speedup 1.0000x reference)
"""Trainium2 Bass kernel for nn_BaselineAttn (LoRA QKV + ALiBi causal attention).

Sharding: 8 cores SPMD, no collectives. Core c = (b, g): batch b = c // 4,
head group g = c % 4 handling heads [g, 4+g, 8+g, 12+g].

Host prep: LoRA folded into weights (W' = W + 2 A@B); x and weights
pre-transposed/sliced per core; partial outputs summed on host.

Device design (fp16 operands, fp32 PSUM):
  - feature-major x^T on chip -> q^T, k^T feature-major and v token-major
    from the same x^T; zero on-chip transposes.
  - attention in the S^T (key-major) orientation, with the ALiBi bias
    folded into the S matmul itself: k^T tiles carry a 65th contraction row
    holding -8*slope_h*k and q^T tiles carry a matching ones row, so
    S^T = k_ext^T.T @ q_ext^T already includes the softmax-shifted ALiBi
    term and the ScalarE exp needs no per-partition bias. One activation
    covers BOTH slots of a pair (their S tiles share one 2-bank PSUM
    tile), halving the exp op count.
      P^T = exp(S^T/8), causal: only the 128-wide diagonal block of each
      diagonal-band tile is partially masked -> one shared [128,128]
      triangular mask multiply (GpSimd) per diagonal tile; the OT matmul of
      a diagonal tile is split into mask-free columns (issued right after
      exp) + the masked 128-block, hiding the mask latency.
      O^T += vext.T @ P^T where vext = [ones*64 | v]: the ones block
      replicates the softmax denominator onto partitions 0..63 so the DVE
      fast-reciprocal reads it straight from PSUM (base partition 0);
      normalize is recip + one DVE mul.
      out-partial = O^T_norm.T @ Wp'^T-slice, written f16 (host sums f32).
  - ALiBi gives key k weight exp(-slope_h*k); p is stored f16 whose
    subnormal floor is ~e^-17, so tiles beyond slope_h*128*kt > ~16 are
    exactly zero anyway and are skipped: SNKT = [1, 2, 8, 16].
  - fused per-chunk pipeline: QKV projection of chunk c, then attention and
    output projection of q-chunk c, all sharing ONE PSUM pool (3 x 2-bank
    slots for qkv accumulators and S~tiles, 2 x 1-bank for O~/proj), so
    attention exps (ScalarE) overlap later chunks' projection matmuls (PE)
    instead of the phases serializing on PSUM bank reuse.
  - inputs arrive via a few large contiguous DMAs (each dma_start costs
    ~600ns of HWDGE sequencer issue time), first-matmul working set first
    and split in half, across the sync/scalar HWDGE rings + SWDGE for tiny
    loads; a short PE warm-up chain and a dummy exp (ACT table load) run
    inside the DMA window.
"""

import math

import numpy as np

E = 1024
H = 16
DH = 64
T = 2048
BATCH = 2
LORA_S = 2.0
NKT = T // 128          # 16 key tiles of 128
SNKT = [1, 2, 8, 16]    # per-slot key-tile caps (max over cores per slot)
NQC = 4                 # q chunks of 512
N_WARMUP = 8            # warm-up matmuls (N=256) to pre-warm the PE clock

_NC_CACHE = {}


def _slopes():
    start = 2 ** (-2 ** (-(math.log2(H) - 3)))
    return np.array([start * start**i for i in range(H)], dtype=np.float64)


def _smin(tt):
    """Lowest slot that still needs key-tile tt."""
    for s in range(4):
        if tt < SNKT[s]:
            return s
    return 4


def _build_nc():
    """Build the single SPMD Bass program (shared by all 8 cores)."""
    if "nc" in _NC_CACHE:
        return _NC_CACHE["nc"]

    from concourse.bacc import Bacc
    import concourse.tile as tile
    from concourse import mybir

    f16 = mybir.dt.float16
    f32 = mybir.dt.float32
    EXP = mybir.ActivationFunctionType.Exp

    nc = Bacc()

    xc_d = [nc.dram_tensor(f"xc{c}", [128, 8, 512], f16, kind="ExternalInput")
            for c in range(NQC)]
    wq_d = nc.dram_tensor("wqT", [128, 8, 256], f16, kind="ExternalInput")
    wkv_d = nc.dram_tensor("wkvT", [128, 8, 512], f16, kind="ExternalInput")
    wp_d = nc.dram_tensor("wpT", [128, 2, 1024], f16, kind="ExternalInput")
    kbias_d = nc.dram_tensor("kbias", [1, 4, 2048], f16, kind="ExternalInput")
    qones_d = nc.dram_tensor("qones", [1, 4, 2048], f16, kind="ExternalInput")
    mask_d = nc.dram_tensor("masks", [128, 128], f16, kind="ExternalInput")
    out_d = nc.dram_tensor("outp", [T, E], f16, kind="ExternalOutput")
    scr_d = nc.dram_tensor("scratch", [128, 8], f16, kind="ExternalOutput")

    with tile.TileContext(nc) as tc:
        with (
            tc.tile_pool(name="persist", bufs=1) as pp,
            tc.tile_pool(name="ptpool", bufs=8) as ptp,
            tc.tile_pool(name="onorm", bufs=4) as onp,
            tc.tile_pool(name="rpool", bufs=4) as rp,
            tc.tile_pool(name="outsb", bufs=6) as osp,
        ):
            # ---- PE warm-up source, independent of DMA ----
            wm_sb = pp.tile([128, 256], f16, name="wm_sb")
            nc.gpsimd.memset(wm_sb, 0.5)
            wexp = pp.tile([128, 8], f16, name="wexp")
            scr_sb = pp.tile([128, 8], f16, name="scr_sb")

            # ---- input loads: few, large, contiguous; first-MM set first --
            xT = [pp.tile([128, 8, 512], f16, name=f"xT{c}") for c in range(NQC)]
            wq_sb = pp.tile([128, 8, 256], f16, name="wq_sb")
            wkv_sb = pp.tile([128, 8, 512], f16, name="wkv_sb")
            # q_ext/k_ext: [65, slot, keys]: rows 0:64 features, row 64 =
            # ones (q) / -8*slope*k ALiBi bias row (k).
            qts = pp.tile([65, 4, 2048], f16, name="qts")
            kts = pp.tile([65, 4, 2048], f16, name="kts")

            # both HWDGE rings stream x in parallel: the scalar ring is idle
            # after the small wq load, so chunk 1 rides it.
            nc.sync.dma_start(out=xT[0][:, :, 0:256], in_=xc_d[0][:, :, 0:256])
            nc.scalar.dma_start(out=wq_sb[:, :, 0:128], in_=wq_d[:, :, 0:128])
            nc.sync.dma_start(out=wkv_sb[:, :, 0:256], in_=wkv_d[:, :, 0:256])
            nc.scalar.dma_start(out=wq_sb[:, :, 128:256], in_=wq_d[:, :, 128:256])
            nc.sync.dma_start(out=wkv_sb[:, :, 256:512], in_=wkv_d[:, :, 256:512])
            nc.sync.dma_start(out=xT[0][:, :, 256:512], in_=xc_d[0][:, :, 256:512])
            nc.scalar.dma_start(out=xT[1], in_=xc_d[1][:, :, :])
            nc.sync.dma_start(out=xT[2], in_=xc_d[2][:, :, :])
            nc.sync.dma_start(out=xT[3], in_=xc_d[3][:, :, :])
            nc.gpsimd.dma_start(out=kts[64:65, :, :], in_=kbias_d[:, :, :])
            nc.gpsimd.dma_start(out=qts[64:65, :, :], in_=qones_d[:, :, :])
            mask_sb = pp.tile([128, 128], f16, name="mask")
            nc.gpsimd.dma_start(out=mask_sb, in_=mask_d[:, :])
            wp_sb = pp.tile([128, 2, 1024], f16, name="wp_sb")
            # dummy exp hoists the ~1.3us ACT table load into the DMA window
            # (after the scalar-ring DMA triggers so it doesn't delay them).
            nc.scalar.activation(out=wexp, in_=wm_sb[:, 0:8], func=EXP,
                                 bias=0.0, scale=0.125)

            # warm-up matmuls cover the PE-idle DMA window so the HAM clock
            # gate is released before real work; results are dead.
            with tc.tile_pool(name="wups", bufs=1, space="PSUM") as wups:
                wacc = wups.tile([128, 512], f32, name="wacc")
                for _ in range(N_WARMUP):
                    nc.tensor.matmul(wacc[:, 0:256], wm_sb[:, 0:128], wm_sb,
                                     start=True, stop=True)
                # tiny live sink so the chain can't be dead-code-eliminated
                nc.vector.tensor_copy(out=scr_sb, in_=wacc[:, 0:8])
                nc.sync.dma_start(out=scr_d[:, :], in_=scr_sb)

            # vext[tt]: [128 keys, slot, 128]: cols 0:64 = ones (denominator
            # replicas at base partition 0), cols 64:128 = v.
            vext = []
            for tt in range(NKT):
                v_t = pp.tile([128, 4, 128], f16, name=f"vext{tt}")
                nc.gpsimd.memset(v_t[:, :, 0:64], 1.0)
                vext.append(v_t)

            ncopy = 0  # round-robin Act/DVE for qk PSUM->SBUF copies
            nosb = 0

            # ---- fused per-chunk pipeline ----
            with tc.tile_pool(name="bigp", bufs=3, space="PSUM") as bigp, \
                 tc.tile_pool(name="spool", bufs=2, space="PSUM") as spool:

                def palloc(name):
                    """One [128,512] f32 accumulator in a shared 2-bank slot."""
                    t = bigp.tile([128, 2, 512], f32, tag="big", name=name)
                    return t[:, 0, :]

                for ncu in range(NQC):
                    # ---------- QKV projection for chunk ncu ----------
                    with nc.named_scope(f"qkv_{ncu}"):
                        # chunk 0 runs in two 256-query halves so the first
                        # matmuls only depend on the first half-DMAs.
                        spans = [(0, 256), (256, 512)] if ncu == 0 else [(0, 512)]
                        for a, b in spans:
                            for which, dst in (("q", qts), ("k", kts)):
                                for mt in range(2):
                                    if which == "k" and mt == 0:
                                        if ncu >= 1:
                                            continue  # slots 0,1: keys < 256
                                        ka, kb = a, min(b, 256)
                                        if ka >= kb:
                                            continue
                                    else:
                                        ka, kb = a, b
                                    nw = kb - ka
                                    acc = palloc(f"qk_{which}{mt}_{ncu}_{ka}")
                                    for kt in range(8):
                                        w_sb = (wq_sb if which == "q" else wkv_sb)
                                        nc.tensor.matmul(
                                            acc[:, 0:nw],
                                            w_sb[:, kt, mt * 128:(mt + 1) * 128],
                                            xT[ncu][:, kt, ka:kb],
                                            start=(kt == 0), stop=(kt == 7),
                                        )
                                    for half in range(2):
                                        s = 2 * mt + half
                                        if which == "k" and s == 2 and ncu >= 2:
                                            continue  # slot 2: keys < 1024
                                        ncopy += 1
                                        eng = (nc.scalar.copy if ncopy % 2 else
                                               nc.vector.tensor_copy)
                                        eng(out=dst[0:64, s,
                                                    ncu * 512 + ka:ncu * 512 + kb],
                                            in_=acc[64 * half:64 * half + 64, 0:nw])
                        if ncu == 0:
                            # wp not needed until the first output projection
                            # (~25us): keep its 0.5MB out of the critical
                            # early-DMA window.
                            nc.scalar.dma_start(out=wp_sb, in_=wp_d[:, :, :])
                        for tt in range(4 * ncu, 4 * ncu + 4):
                            s0 = _smin(tt)
                            if s0 >= 4:
                                continue
                            nw = (4 - s0) * 64
                            acc = palloc(f"vacc{tt}")
                            for kt in range(8):
                                nc.tensor.matmul(
                                    acc[:, 0:nw],
                                    xT[ncu][:, kt, (tt % 4) * 128:(tt % 4 + 1) * 128],
                                    wkv_sb[:, kt, 256 + s0 * 64:512],
                                    start=(kt == 0), stop=(kt == 7),
                                )
                            nc.vector.tensor_copy(
                                out=vext[tt][:, s0:4, 64:128],
                                in_=acc[:, 0:nw].rearrange("p (s d) -> p s d", d=64))

                    # ---------- attention + output proj for q-chunk ncu ----
                    qc = ncu
                    on_tiles = [onp.tile([128, 512], f16, tag="on",
                                         name=f"on_{qc}_{p}") for p in range(2)]
                    # small pair (slots 0,1) first: it clears quickly and its
                    # normalize hides under the big pair's chain.
                    for pair in (0, 1):
                        sA, sB = 2 * pair, 2 * pair + 1
                        nktA = min(SNKT[sA], 4 * qc + 4)
                        nktB = min(SNKT[sB], 4 * qc + 4)
                        # units: ("AB", kt) = slots A+B same kt in one 2-bank
                        # st tile; ("BB", kt) = slot B tiles kt, kt+1 (both
                        # full); ("B", kt) = slot B solo.
                        units = []
                        for kt in range(nktA):
                            units.append(("AB", kt))
                        rem = list(range(nktA, nktB))
                        i = 0
                        while i < len(rem):
                            kt = rem[i]
                            if (kt < 4 * qc and i + 1 < len(rem)
                                    and rem[i + 1] < 4 * qc):
                                units.append(("BB", kt))
                                i += 2
                            else:
                                units.append(("B", kt))
                                i += 1
                        ot = {s: spool.tile([128, 512], f32, tag="ot",
                                            name=f"ot_{qc}_{s}")
                              for s in (sA, sB)}
                        started = {sA: False, sB: False}

                        def unit_tiles(u):
                            kind, kt = u
                            if kind == "AB":
                                return [(sA, kt, 0), (sB, kt, 1)]
                            if kind == "BB":
                                return [(sB, kt, 0), (sB, kt + 1, 1)]
                            return [(sB, kt, 0)]
                        n_ot = {sA: nktA, sB: nktB}
                        done_ot = {sA: 0, sB: 0}
                        with nc.named_scope(f"attn_q{qc}_p{pair}"):
                            for u in units:
                                st2 = bigp.tile([128, 2, 512], f32, tag="big",
                                                name=f"st_{qc}_{pair}_{u[1]}")
                                p2 = ptp.tile([128, 2, 512], f16, tag="pt",
                                              name=f"pt_{qc}_{pair}_{u[1]}")
                                tl = unit_tiles(u)
                                j0s = [(kt - 4 * qc) * 128 if kt >= 4 * qc else 0
                                       for (_, kt, _) in tl]
                                for (s, kt, h), j0 in zip(tl, j0s):
                                    nc.tensor.matmul(
                                        st2[:, h, j0:512],
                                        kts[0:65, s, kt * 128:(kt + 1) * 128],
                                        qts[0:65, s, qc * 512 + j0:(qc + 1) * 512],
                                        start=True, stop=True,
                                    )
                                # one exp for the whole unit (no bias needed)
                                j0 = j0s[0]
                                if len(tl) == 2:
                                    nc.scalar.activation(
                                        out=p2[:, :, j0:512], in_=st2[:, :, j0:512],
                                        func=EXP, bias=0.0, scale=0.125)
                                else:
                                    nc.scalar.activation(
                                        out=p2[:, 0, j0:512], in_=st2[:, 0, j0:512],
                                        func=EXP, bias=0.0, scale=0.125)
                                for (s, kt, h), j0 in zip(tl, j0s):
                                    if kt >= 4 * qc:
                                        nc.gpsimd.tensor_mul(
                                            out=p2[:, h, j0:j0 + 128],
                                            in0=p2[:, h, j0:j0 + 128],
                                            in1=mask_sb,
                                        )
                                # OT: mask-free columns first (no mask wait),
                                # then the masked 128-block.
                                for (s, kt, h), j0 in zip(tl, j0s):
                                    diag = kt >= 4 * qc
                                    done_ot[s] += 1
                                    last = done_ot[s] == n_ot[s]
                                    segs = []
                                    if diag:
                                        if j0 + 128 < 512:
                                            segs.append((j0 + 128, 512, False))
                                        segs.append((j0, j0 + 128, True))
                                    else:
                                        segs.append((0, 512, False))
                                    for si, (aa, bb, _) in enumerate(segs):
                                        nc.tensor.matmul(
                                            ot[s][:, aa:bb],
                                            vext[kt][:, s, :],
                                            p2[:, h, aa:bb],
                                            start=not started[s],
                                            stop=last and si == len(segs) - 1,
                                        )
                                        started[s] = True
                                    if not last:
                                        continue
                                    # this slot's chain is done: normalize now
                                    # (fast-reciprocal straight from PSUM rows
                                    # 0:64, then one DVE mul) so its ot bank
                                    # frees while the other slot still runs.
                                    r0 = 64 * (s % 2)
                                    rec = rp.tile([64, 512], f32, tag="rec",
                                                  name=f"rec_{qc}_{s}")
                                    nc.vector.reciprocal_approx_fast(
                                        out=rec, in_=ot[s][0:64, :])
                                    nc.vector.tensor_mul(
                                        out=on_tiles[pair][r0:r0 + 64, :],
                                        in0=ot[s][64:128, :],
                                        in1=rec,
                                    )
                    with nc.named_scope(f"proj_q{qc}"):
                        for tloc in range(4):
                            tt = qc * 4 + tloc
                            osb = osp.tile([128, 1024], f16, tag="osb",
                                           name=f"osb_{tt}")
                            # last chunk: S~tiles are done, so the big pool's
                            # 6 banks are free -> deeper pacc pipeline for a
                            # shorter tail.
                            p_t2 = (bigp.tile([128, 2, 512], f32, tag="big",
                                              name=f"paccs_{tt}")
                                    if qc == NQC - 1 else None)
                            for ech in range(2):
                                pacc = (p_t2[:, ech, :] if p_t2 is not None else
                                        spool.tile([128, 512], f32, tag="ot",
                                                   name=f"pacc_{tt}_{ech}"))
                                for pt_i in (0, 1):
                                    nc.tensor.matmul(
                                        pacc,
                                        on_tiles[pt_i][:, tloc * 128:(tloc + 1) * 128],
                                        wp_sb[:, pt_i, ech * 512:(ech + 1) * 512],
                                        start=(pt_i == 0), stop=(pt_i == 1),
                                    )
                                nosb += 1
                                if nosb % 4 == 0:
                                    nc.scalar.copy(
                                        out=osb[:, ech * 512:(ech + 1) * 512],
                                        in_=pacc)
                                else:
                                    nc.vector.tensor_copy(
                                        out=osb[:, ech * 512:(ech + 1) * 512],
                                        in_=pacc)
                            nc.sync.dma_start(
                                out=out_d[tt * 128:(tt + 1) * 128, :], in_=osb)

    nc.finalize()
    _NC_CACHE["nc"] = nc
    return nc


def _prep_core_inputs(x, Wq, Aq, Bq, Wk, Ak, Bk, Wv, Av, Bv, Wp):
    """Host-side prep: LoRA fold, transposes, per-core slices."""
    slopes = _slopes()
    wq_m = Wq.astype(np.float64) + LORA_S * (Aq.astype(np.float64) @ Bq.astype(np.float64))
    wk_m = Wk.astype(np.float64) + LORA_S * (Ak.astype(np.float64) @ Bk.astype(np.float64))
    wv_m = Wv.astype(np.float64) + LORA_S * (Av.astype(np.float64) @ Bv.astype(np.float64))

    # shared [128,128] triangular mask: within a diagonal 128-block,
    # key-in-tile p is valid for local col j iff p <= j.
    p_i = np.arange(128)[:, None]
    j_i = np.arange(128)[None, :]
    masks = np.ascontiguousarray((p_i <= j_i).astype(np.float16))
    qones = np.ones((1, 4, 2048), dtype=np.float16)

    in_maps = []
    for c in range(8):
        b, g = divmod(c, 4)
        heads = [g, 4 + g, 8 + g, 12 + g]
        rows = np.concatenate([np.arange(h * DH, (h + 1) * DH) for h in heads])
        xT = x[b].T.astype(np.float16)          # [E, T]
        wqT = wq_m[rows, :].T.astype(np.float16)         # [E, 256]
        wkvT = np.concatenate(
            [wk_m[rows, :].T, wv_m[rows, :].T], axis=1).astype(np.float16)  # [E,512]
        wpT = Wp[:, rows].T.astype(np.float16)           # [256, E]
        kbias = np.zeros((1, 4, 2048), dtype=np.float16)
        for s, h in enumerate(heads):
            kbias[0, s, :] = (-8.0 * slopes[h] * np.arange(2048)).astype(np.float16)
        im = {
            "wqT": np.ascontiguousarray(
                wqT.reshape(8, 128, 256).transpose(1, 0, 2)),
            "wkvT": np.ascontiguousarray(
                wkvT.reshape(8, 128, 512).transpose(1, 0, 2)),
            "wpT": np.ascontiguousarray(
                wpT.reshape(2, 128, 1024).transpose(1, 0, 2)),
            "kbias": kbias, "qones": qones, "masks": masks,
        }
        for cch in range(NQC):
            im[f"xc{cch}"] = np.ascontiguousarray(
                xT[:, cch * 512:(cch + 1) * 512]
                .reshape(8, 128, 512).transpose(1, 0, 2))
        in_maps.append(im)
    return in_maps


def _run(in_maps, trace=False, **kw):
    from concourse.bass_utils import run_bass_kernel_spmd
    nc = _build_nc()
    return run_bass_kernel_spmd(nc, in_maps, core_ids=list(range(8)), trace=trace, **kw)


def kernel(x, Wq, Aq, Bq, Wk, Ak, Bk, Wv, Av, Bv, Wp):
    in_maps = _prep_core_inputs(x, Wq, Aq, Bq, Wk, Ak, Bk, Wv, Av, Bv, Wp)
    res = _run(in_maps)
    out = np.zeros((BATCH, T, E), dtype=np.float32)
    for c in range(8):
        out[c // 4] += res.results[c]["outp"].astype(np.float32)
    return out
